# revision 5
# baseline (speedup 1.0000x reference)
"""Trainium2 Bass kernel for NodeUpdateNetwork-style GNN message passing.

out = relu(BN((x + ((sim - dsim) @ x) / N) @ W.T))  with sync-BN over (B, N).

Sharding: data-parallel over batch across 8 NeuronCores (2 batches/core);
W/gamma/beta replicated; BN statistics all-reduced across cores in-kernel.

The on-chip pipeline keeps the feature dimension on partitions ("transposed
space") so that BN reduces run along the free axis and the BN+ReLU apply is a
single per-partition scalar-engine activation:
  - stream sim/dsim row-stripes [128, N] fp32 (contiguous HBM reads)
  - DVE: diff = sim - dsim (bf16 out)
  - PE: transpose 128x128 diff tiles (identity matmul, bf16)
  - PE: aggT[f, i] += (x/N)[j, f]^T-contracted with diffT[j, i]
  - yT = aggT + xT ; zT = W @ yT (fp32) ; BN stats; AllReduce; apply; untranspose

v2: software-pipelined across reps — the post-collective tail of pass r-1
(BN math, ReLU apply, untranspose, output stores) is emitted after the first
few stream groups of pass r, so in steady state the AllReduce latency and the
apply phase hide completely under the next pass's edge streaming. Queue
hygiene keeps the SP ring free for edge loads: collective bounce DMAs ride
the gpsimd (Pool) queue, output stores ride the scalar (ACT) HWDGE ring.
"""

import sys

if "/opt/trn_rl_repo" not in sys.path:
    sys.path.insert(0, "/opt/trn_rl_repo")

import numpy as np
import ml_dtypes

import concourse.bacc as bacc
import concourse.mybir as mybir
import concourse.tile as tile
from concourse.bass_utils import run_bass_kernel_spmd

N_CORES = 8
B, N, F = 16, 2048, 64
B_PC = B // N_CORES
BN_EPS = 1e-5
BF16 = mybir.dt.bfloat16
F32 = mybir.dt.float32


def build_nc(
    n_cores=N_CORES, b_pc=B_PC, n=N, f=F, b_total=None, reps=1, mode="full",
    tail_at=3,
):
    """Build the per-core Bass program (same program on every core).

    reps > 1 unrolls the whole computation multiple times (for timing-slope
    measurements: HW time per pass = (t(reps=R) - t(reps=1)) / (R - 1)).
    mode: "full" | "nocc" (collective replaced by local dram copy, timing
    only) | "dmaonly" (edge stream loads only, timing only).
    tail_at: how many stream groups of pass r are emitted before the tail of
    pass r-1 (software pipelining depth for hiding the collective latency).
    """
    assert f == 64
    if b_total is None:
        b_total = n_cores * b_pc
    NT = n // 128                      # number of 128-wide j tiles
    IB = min(4, NT)                    # i-blocks (128 rows) per group
    GW = IB * 128                      # group width along i (<= 512)
    NG = n // GW                       # groups per batch
    inv_count = 1.0 / (b_total * n)

    nc = bacc.Bacc(
        "TRN2", target_bir_lowering=False, debug=False, num_devices=n_cores
    )

    edge = nc.dram_tensor("edge", [b_pc, 2, n, n], F32, kind="ExternalInput").ap()
    xt = nc.dram_tensor("xt", [b_pc, f, n], F32, kind="ExternalInput").ap()
    # xn is pre-laid-out host-side as [128, NT, f] per batch (contiguous DMA)
    xn = nc.dram_tensor("xn", [b_pc, 128, NT, f], BF16, kind="ExternalInput").ap()
    wt = nc.dram_tensor("wt", [f, f], F32, kind="ExternalInput").ap()
    gamma = nc.dram_tensor("gamma", [f, 1], F32, kind="ExternalInput").ap()
    beta = nc.dram_tensor("beta", [f, 1], F32, kind="ExternalInput").ap()
    i128 = nc.dram_tensor("i128", [128, 128], BF16, kind="ExternalInput").ap()
    i64 = nc.dram_tensor("i64", [f, f], F32, kind="ExternalInput").ap()
    out = nc.dram_tensor("out", [b_pc, n, f], F32, kind="ExternalOutput").ap()

    with tile.TileContext(nc) as tc:
        with (
            tc.tile_pool(name="const", bufs=1) as cpool,
            tc.tile_pool(name="io", bufs=2) as iopool,
            tc.tile_pool(name="zq", bufs=2 * b_pc) as zqpool,
            tc.tile_pool(name="stream", bufs=3) as spool,
            tc.tile_pool(name="blk", bufs=2) as bpool,
            tc.tile_pool(name="psum", bufs=2, space="PSUM") as ppool,
            tc.tile_pool(name="dram", bufs=2, space="DRAM") as dpool,
        ):
            # --- constants ---
            i128_sb = cpool.tile([128, 128], BF16)
            nc.sync.dma_start(i128_sb[:], i128[:])
            i64_sb = cpool.tile([f, f], F32)
            nc.sync.dma_start(i64_sb[:], i64[:])
            wt_sb = cpool.tile([f, f], F32)
            nc.sync.dma_start(wt_sb[:], wt[:])
            gamma_sb = cpool.tile([f, 1], F32)
            nc.sync.dma_start(gamma_sb[:], gamma[:])
            beta_sb = cpool.tile([f, 1], F32)
            nc.sync.dma_start(beta_sb[:], beta[:])

            def dma_only_pass():
                # dummy consumer so bacc/walrus DCE keeps the loads
                dum = cpool.tile([128, 2], F32, tag="dum")
                for b in range(b_pc):
                    for g in range(NG):
                        for ib in range(IB):
                            i0 = g * GW + ib * 128
                            sim_sb = spool.tile([128, n], F32, tag="sim")
                            nc.sync.dma_start(
                                sim_sb[:], edge[b, 0, i0 : i0 + 128, :]
                            )
                            dsim_sb = spool.tile([128, n], F32, tag="dsim")
                            nc.sync.dma_start(
                                dsim_sb[:], edge[b, 1, i0 : i0 + 128, :]
                            )
                            nc.vector.reduce_sum(
                                dum[:, 0:1], sim_sb[:, 0:4],
                                axis=mybir.AxisListType.X,
                            )
                            nc.vector.reduce_sum(
                                dum[:, 1:2], dsim_sb[:, 0:4],
                                axis=mybir.AxisListType.X,
                            )
                nc.sync.dma_start(out[0, 0:128, 0:2], dum[:])

            def emit_prefetch(st):
                """Load per-batch node features for a pass (SP ring)."""
                for b in range(b_pc):
                    xt_sb = iopool.tile([f, n], F32, tag="xt", bufs=b_pc)
                    nc.sync.dma_start(xt_sb[:], xt[b])
                    xn_sb = iopool.tile([128, NT, f], BF16, tag="xn", bufs=b_pc)
                    nc.sync.dma_start(xn_sb[:], xn[b])
                    zq_sb = zqpool.tile([f, n], F32, tag="zq")
                    st["xt"].append(xt_sb)
                    st["xn"].append(xn_sb)
                    st["zq"].append(zq_sb)
                st["stats"] = cpool.tile(
                    [f, b_pc * NG, 2], F32, tag="stats", bufs=2, name="stats_sb"
                )

            def emit_group(st, b, g):
                xt_sb, xn_sb, zq_sb = st["xt"][b], st["xn"][b], st["zq"][b]
                # --- load group stripes and subtract ---
                diff_all = bpool.tile([128, IB, n], BF16, tag="diff")
                for ib in range(IB):
                    i0 = g * GW + ib * 128
                    sim_sb = spool.tile([128, n], F32, tag="sim")
                    nc.sync.dma_start(sim_sb[:], edge[b, 0, i0 : i0 + 128, :])
                    dsim_sb = spool.tile([128, n], F32, tag="dsim")
                    nc.sync.dma_start(dsim_sb[:], edge[b, 1, i0 : i0 + 128, :])
                    nc.vector.tensor_sub(
                        diff_all[:, ib, :], sim_sb[:], dsim_sb[:]
                    )

                # --- transpose diff tiles: dT[j, i] = diff[i, j] ---
                dT_all = bpool.tile([128, NT, GW], BF16, tag="dT")
                for jt in range(NT):
                    tpsum = ppool.tile([128, GW], BF16, tag="tpsum")
                    for ib in range(IB):
                        nc.tensor.transpose(
                            tpsum[:, ib * 128 : (ib + 1) * 128],
                            diff_all[:, ib, jt * 128 : (jt + 1) * 128],
                            i128_sb[:],
                        )
                    if jt % 2 == 0:
                        nc.vector.tensor_copy(dT_all[:, jt, :], tpsum[:])
                    else:
                        nc.scalar.copy(dT_all[:, jt, :], tpsum[:])

                # --- aggT[f, i] = sum_j (x/N)[j, f] * diff[i, j] ---
                aggT = ppool.tile([f, GW], F32, tag="agg")
                for jt in range(NT):
                    nc.tensor.matmul(
                        aggT[:],
                        xn_sb[:, jt, :],
                        dT_all[:, jt, :],
                        start=(jt == 0),
                        stop=(jt == NT - 1),
                    )

                # --- yT = aggT + xT ; zT = W @ yT ---
                yT_sb = bpool.tile([f, GW], F32, tag="yT")
                nc.vector.tensor_add(
                    yT_sb[:], aggT[:], xt_sb[:, g * GW : (g + 1) * GW]
                )
                zT = ppool.tile([f, GW], F32, tag="zT")
                nc.tensor.matmul(
                    zT[:], wt_sb[:], yT_sb[:], start=True, stop=True
                )

                # stash z and accumulate BN partial sums
                nc.scalar.copy(zq_sb[:, g * GW : (g + 1) * GW], zT[:])
                gi = b * NG + g
                nc.vector.reduce_sum(
                    st["stats"][:, gi, 0:1], zT[:], axis=mybir.AxisListType.X
                )
                sq_sb = bpool.tile([f, GW], F32, tag="sq")
                nc.scalar.activation(
                    sq_sb[:],
                    zT[:],
                    mybir.ActivationFunctionType.Square,
                    accum_out=st["stats"][:, gi, 1:2],
                )

            def emit_stats_cc(st):
                """Local stats -> global stats (sync-BN all-reduce).

                All bounce DMAs + the collective ride the gpsimd (Pool)
                queue so no stream engine ever waits on the collective.
                """
                stats_loc = cpool.tile([f, 2], F32, tag="stats_loc", bufs=2)
                nc.vector.reduce_sum(
                    stats_loc[:],
                    st["stats"][:].rearrange("p g s -> p s g"),
                    axis=mybir.AxisListType.X,
                )
                cc_in = dpool.tile([f, 2], F32, tag="cc_in")
                cc_out = dpool.tile([f, 2], F32, tag="cc_out")
                nc.gpsimd.dma_start(cc_in[:], stats_loc[:])
                if mode == "nocc":
                    nc.gpsimd.dma_start(cc_out[:], cc_in[:])
                else:
                    nc.gpsimd.collective_compute(
                        "AllReduce",
                        mybir.AluOpType.add,
                        replica_groups=[list(range(n_cores))],
                        ins=[cc_in.opt()],
                        outs=[cc_out.opt()],
                    )
                stats_tot = cpool.tile(
                    [f, 2], F32, tag="stats_tot", bufs=2, name="stats_tot"
                )
                nc.gpsimd.dma_start(stats_tot[:], cc_out[:])
                st["stats_tot"] = stats_tot

            def emit_tail(st):
                """BN math + ReLU apply + untranspose + store (pass st)."""
                stats_tot = st["stats_tot"]
                sc_sb = cpool.tile([f, 12], F32, tag="sc", bufs=2)
                mean = sc_sb[:, 0:1]
                es2 = sc_sb[:, 1:2]
                msq = sc_sb[:, 2:3]
                var = sc_sb[:, 3:4]
                std = sc_sb[:, 4:5]
                rstd = sc_sb[:, 5:6]
                scl = sc_sb[:, 6:7]
                tmp = sc_sb[:, 7:8]
                shf = sc_sb[:, 8:9]
                varp = sc_sb[:, 9:10]
                nc.vector.tensor_scalar_mul(mean, stats_tot[:, 0:1], inv_count)
                nc.vector.tensor_scalar_mul(es2, stats_tot[:, 1:2], inv_count)
                nc.vector.tensor_mul(msq, mean, mean)
                nc.vector.tensor_sub(var, es2, msq)
                nc.vector.tensor_scalar_add(varp, var, BN_EPS)
                nc.scalar.activation(std, varp, mybir.ActivationFunctionType.Sqrt)
                nc.vector.reciprocal(rstd, std)
                nc.vector.tensor_mul(scl, gamma_sb[:], rstd)
                nc.vector.tensor_mul(tmp, mean, scl)
                nc.vector.tensor_sub(shf, beta_sb[:], tmp)

                # --- apply BN+ReLU, untranspose, store (ACT HWDGE ring) ---
                for b in range(b_pc):
                    zr_sb = iopool.tile([f, n], F32, tag="zr")
                    nc.scalar.activation(
                        zr_sb[:],
                        st["zq"][b][:],
                        mybir.ActivationFunctionType.Relu,
                        bias=shf,
                        scale=scl,
                    )
                    out_sb = iopool.tile([128, NT, f], F32, tag="out")
                    for c in range(NT):
                        bpsum = ppool.tile([128, f], F32, tag="bpsum")
                        nc.tensor.transpose(
                            bpsum[:], zr_sb[:, c * 128 : (c + 1) * 128], i64_sb[:]
                        )
                        nc.vector.tensor_copy(out_sb[:, c, :], bpsum[:])
                    nc.scalar.dma_start(
                        out[b].rearrange("(t p) f -> p t f", p=128), out_sb[:]
                    )

            if mode == "dmaonly":
                for _ in range(reps):
                    dma_only_pass()
            else:
                groups = [(b, g) for b in range(b_pc) for g in range(NG)]
                prev = None
                for _ in range(reps):
                    st = {"xt": [], "xn": [], "zq": []}
                    emit_prefetch(st)
                    for idx, (b, g) in enumerate(groups):
                        emit_group(st, b, g)
                        if idx + 1 == tail_at and prev is not None:
                            emit_tail(prev)
                            prev = None
                    emit_stats_cc(st)
                    if prev is not None:
                        # tail_at > number of groups: emit late
                        emit_tail(prev)
                    prev = st
                emit_tail(prev)

    nc.compile()
    return nc


def make_in_maps(node_feats, edge_feats, W, gamma, beta, n_cores=N_CORES):
    b, n, f = node_feats.shape
    b_pc = b // n_cores
    nt = n // 128
    node_feats = np.asarray(node_feats, dtype=np.float32)
    edge_feats = np.asarray(edge_feats, dtype=np.float32)
    wt = np.ascontiguousarray(np.asarray(W, dtype=np.float32).T)
    gamma = np.asarray(gamma, dtype=np.float32).reshape(f, 1)
    beta = np.asarray(beta, dtype=np.float32).reshape(f, 1)
    i128 = np.eye(128, dtype=np.float32).astype(ml_dtypes.bfloat16)
    i64 = np.eye(f, dtype=np.float32)
    in_maps = []
    for c in range(n_cores):
        sl = slice(c * b_pc, (c + 1) * b_pc)
        xs = node_feats[sl]
        xnp = (xs / np.float32(n)).astype(ml_dtypes.bfloat16)
        # [b_pc, n, f] -> [b_pc, 128, nt, f]: partition p holds row jt*128+p
        xnp = np.ascontiguousarray(
            xnp.reshape(b_pc, nt, 128, f).transpose(0, 2, 1, 3)
        )
        in_maps.append(
            {
                "edge": edge_feats[sl],
                "xt": np.ascontiguousarray(xs.transpose(0, 2, 1)),
                "xn": xnp,
                "wt": wt,
                "gamma": gamma,
                "beta": beta,
                "i128": i128,
                "i64": i64,
            }
        )
    return in_maps


_NC_CACHE = {}


def _get_nc(key=(N_CORES, B_PC, N, F)):
    if key not in _NC_CACHE:
        _NC_CACHE[key] = build_nc(*key)
    return _NC_CACHE[key]


def kernel(node_feats, edge_feats, W, gamma, beta):
    node_feats = np.asarray(node_feats)
    edge_feats = np.asarray(edge_feats)
    b, n, f = node_feats.shape
    n_cores = N_CORES
    b_pc = b // n_cores
    nc = _get_nc((n_cores, b_pc, n, f))
    in_maps = make_in_maps(node_feats, edge_feats, W, gamma, beta, n_cores)
    res = run_bass_kernel_spmd(nc, in_maps, list(range(n_cores)))
    outs = [res.results[c]["out"] for c in range(n_cores)]
    return np.concatenate(outs, axis=0).astype(np.float32)


# revision 9
# speedup vs baseline: 1.1426x; 1.1426x over previous
"""Trainium2 Bass kernel for NodeUpdateNetwork-style GNN message passing.

out = relu(BN((x + ((sim - dsim) @ x) / N) @ W.T))  with sync-BN over (B, N).

Sharding: data-parallel over batch across 8 NeuronCores (2 batches/core);
W/gamma/beta replicated; BN statistics all-reduced across cores in-kernel.

The on-chip pipeline keeps the feature dimension on partitions ("transposed
space") so that BN reduces run along the free axis and the BN+ReLU apply is a
single per-partition scalar-engine activation:
  - stream sim/dsim row-stripes [128, N] fp32 (contiguous HBM reads)
  - DVE: diff = sim - dsim (bf16 out)
  - PE: transpose 128x128 diff tiles (identity matmul, bf16)
  - PE: aggT[f, i] += (x/N)[j, f]^T-contracted with diffT[j, i]
  - yT = aggT + xT ; zT = W @ yT (fp32) ; BN stats; AllReduce; apply; untranspose

v2: software-pipelined across reps — the post-collective tail of pass r-1
(BN math, ReLU apply, untranspose, output stores) is emitted after the first
few stream groups of pass r, so in steady state the AllReduce latency and the
apply phase hide completely under the next pass's edge streaming. Queue
hygiene keeps the SP ring free for edge loads: collective bounce DMAs ride
the gpsimd (Pool) queue, output stores ride the scalar (ACT) HWDGE ring.
"""

import sys

if "/opt/trn_rl_repo" not in sys.path:
    sys.path.insert(0, "/opt/trn_rl_repo")

import numpy as np
import ml_dtypes

import concourse.bacc as bacc
import concourse.mybir as mybir
import concourse.tile as tile
from concourse.bass_utils import run_bass_kernel_spmd

N_CORES = 8
B, N, F = 16, 2048, 64
B_PC = B // N_CORES
BN_EPS = 1e-5
BF16 = mybir.dt.bfloat16
F32 = mybir.dt.float32


def build_nc(
    n_cores=N_CORES, b_pc=B_PC, n=N, f=F, b_total=None, reps=1, mode="full",
    tail_at=3,
):
    """Build the per-core Bass program (same program on every core).

    reps > 1 unrolls the whole computation multiple times (for timing-slope
    measurements: HW time per pass = (t(reps=R) - t(reps=1)) / (R - 1)).
    mode: "full" | "nocc" (collective replaced by local dram copy, timing
    only) | "dmaonly" (edge stream loads only, timing only).
    tail_at: how many stream groups of pass r are emitted before the tail of
    pass r-1 (software pipelining depth for hiding the collective latency).
    """
    assert f == 64
    if b_total is None:
        b_total = n_cores * b_pc
    NT = n // 128                      # number of 128-wide j tiles
    IB = min(4, NT)                    # i-blocks (128 rows) per group
    GW = IB * 128                      # group width along i (<= 512)
    NG = n // GW                       # groups per batch
    inv_count = 1.0 / (b_total * n)

    nc = bacc.Bacc(
        "TRN2", target_bir_lowering=False, debug=False, num_devices=n_cores
    )

    edge = nc.dram_tensor("edge", [b_pc, 2, n, n], F32, kind="ExternalInput").ap()
    xt = nc.dram_tensor("xt", [b_pc, f, n], F32, kind="ExternalInput").ap()
    # xn is pre-laid-out host-side as [128, NT, f] per batch (contiguous DMA)
    xn = nc.dram_tensor("xn", [b_pc, 128, NT, f], BF16, kind="ExternalInput").ap()
    wt = nc.dram_tensor("wt", [f, f], F32, kind="ExternalInput").ap()
    gamma = nc.dram_tensor("gamma", [f, 1], F32, kind="ExternalInput").ap()
    beta = nc.dram_tensor("beta", [f, 1], F32, kind="ExternalInput").ap()
    i128 = nc.dram_tensor("i128", [128, 128], BF16, kind="ExternalInput").ap()
    i64 = nc.dram_tensor("i64", [f, f], F32, kind="ExternalInput").ap()
    out = nc.dram_tensor("out", [b_pc, n, f], F32, kind="ExternalOutput").ap()

    with tile.TileContext(nc) as tc:
        with (
            tc.tile_pool(name="const", bufs=1) as cpool,
            tc.tile_pool(name="io", bufs=2) as iopool,
            tc.tile_pool(name="zq", bufs=2 * b_pc) as zqpool,
            tc.tile_pool(name="stream", bufs=3) as spool,
            tc.tile_pool(name="blk", bufs=2) as bpool,
            tc.tile_pool(name="psum", bufs=2, space="PSUM") as ppool,
            tc.tile_pool(name="dram", bufs=2, space="DRAM") as dpool,
        ):
            # --- constants ---
            i128_sb = cpool.tile([128, 128], BF16)
            nc.sync.dma_start(i128_sb[:], i128[:])
            i64_sb = cpool.tile([f, f], F32)
            nc.sync.dma_start(i64_sb[:], i64[:])
            wt_sb = cpool.tile([f, f], F32)
            nc.sync.dma_start(wt_sb[:], wt[:])
            gamma_sb = cpool.tile([f, 1], F32)
            nc.sync.dma_start(gamma_sb[:], gamma[:])
            beta_sb = cpool.tile([f, 1], F32)
            nc.sync.dma_start(beta_sb[:], beta[:])

            def dma_only_pass():
                # dummy consumer so bacc/walrus DCE keeps the loads
                dum = cpool.tile([128, 2], F32, tag="dum")
                for b in range(b_pc):
                    for g in range(NG):
                        for ib in range(IB):
                            i0 = g * GW + ib * 128
                            sim_sb = spool.tile([128, n], F32, tag="sim")
                            nc.sync.dma_start(
                                sim_sb[:], edge[b, 0, i0 : i0 + 128, :]
                            )
                            dsim_sb = spool.tile([128, n], F32, tag="dsim")
                            nc.sync.dma_start(
                                dsim_sb[:], edge[b, 1, i0 : i0 + 128, :]
                            )
                            nc.vector.reduce_sum(
                                dum[:, 0:1], sim_sb[:, 0:4],
                                axis=mybir.AxisListType.X,
                            )
                            nc.vector.reduce_sum(
                                dum[:, 1:2], dsim_sb[:, 0:4],
                                axis=mybir.AxisListType.X,
                            )
                nc.sync.dma_start(out[0, 0:128, 0:2], dum[:])

            def emit_prefetch(st):
                """Load per-batch node features for a pass (SP ring)."""
                for b in range(b_pc):
                    xt_sb = iopool.tile([f, n], F32, tag="xt", bufs=b_pc)
                    nc.sync.dma_start(xt_sb[:], xt[b])
                    xn_sb = iopool.tile([128, NT, f], BF16, tag="xn", bufs=b_pc)
                    nc.sync.dma_start(xn_sb[:], xn[b])
                    zq_sb = zqpool.tile([f, n], F32, tag="zq")
                    st["xt"].append(xt_sb)
                    st["xn"].append(xn_sb)
                    st["zq"].append(zq_sb)
                st["stats"] = cpool.tile(
                    [f, b_pc * NG, 2], F32, tag="stats", bufs=2, name="stats_sb"
                )

            def emit_group(st, b, g):
                xt_sb, xn_sb, zq_sb = st["xt"][b], st["xn"][b], st["zq"][b]
                # --- load group stripes (sim+dsim paired, 2 MiB) + subtract ---
                diff_all = bpool.tile([128, IB, n], BF16, tag="diff")
                sd_tiles = []
                for ib in range(IB):
                    i0 = g * GW + ib * 128
                    sd_sb = spool.tile([128, 2, n], F32, tag="sd")
                    nc.sync.dma_start(
                        sd_sb[:],
                        edge[b, :, i0 : i0 + 128, :].rearrange("s p n -> p s n"),
                    )
                    sd_tiles.append(sd_sb)
                    nc.vector.tensor_sub(
                        diff_all[:, ib, :], sd_sb[:, 0, :], sd_sb[:, 1, :]
                    )

                # --- transpose diff tiles: dT[j, i] = diff[i, j] ---
                # ib-major so the PE starts on stripe 0 the moment its sub
                # lands (keeps the PE smoothly busy -> HAM stays at 2.4 GHz)
                dT_all = bpool.tile([128, NT, GW], BF16, tag="dT")
                for ib in range(IB):
                    tpsum = ppool.tile([128, NT, 128], BF16, tag="tpsum")
                    for jt in range(NT):
                        nc.tensor.transpose(
                            tpsum[:, jt, :],
                            diff_all[:, ib, jt * 128 : (jt + 1) * 128],
                            i128_sb[:],
                        )
                    if ib % 2 == 0:
                        nc.vector.tensor_copy(
                            dT_all[:, :, ib * 128 : (ib + 1) * 128], tpsum[:]
                        )
                    else:
                        nc.scalar.copy(
                            dT_all[:, :, ib * 128 : (ib + 1) * 128], tpsum[:]
                        )

                # --- aggT[f, i] = sum_j (x/N)[j, f] * diff[i, j] ---
                aggT = ppool.tile([f, GW], F32, tag="agg", bufs=1)
                for jt in range(NT):
                    nc.tensor.matmul(
                        aggT[:],
                        xn_sb[:, jt, :],
                        dT_all[:, jt, :],
                        start=(jt == 0),
                        stop=(jt == NT - 1),
                    )

                # --- yT = aggT + xT ; zT = W @ yT ---
                yT_sb = bpool.tile([f, GW], F32, tag="yT")
                nc.vector.tensor_add(
                    yT_sb[:], aggT[:], xt_sb[:, g * GW : (g + 1) * GW]
                )
                zT = ppool.tile([f, GW], F32, tag="zT", bufs=1)
                nc.tensor.matmul(
                    zT[:], wt_sb[:], yT_sb[:], start=True, stop=True
                )

                # stash z and accumulate BN partial sums
                nc.scalar.copy(zq_sb[:, g * GW : (g + 1) * GW], zT[:])
                gi = b * NG + g
                nc.vector.reduce_sum(
                    st["stats"][:, gi, 0:1], zT[:], axis=mybir.AxisListType.X
                )
                sq_sb = bpool.tile([f, GW], F32, tag="sq")
                nc.scalar.activation(
                    sq_sb[:],
                    zT[:],
                    mybir.ActivationFunctionType.Square,
                    accum_out=st["stats"][:, gi, 1:2],
                )

            def emit_stats_cc(st):
                """Local stats -> global stats (sync-BN all-reduce).

                All bounce DMAs + the collective ride the gpsimd (Pool)
                queue so no stream engine ever waits on the collective.
                """
                stats_loc = cpool.tile([f, 2], F32, tag="stats_loc", bufs=2)
                nc.vector.reduce_sum(
                    stats_loc[:],
                    st["stats"][:].rearrange("p g s -> p s g"),
                    axis=mybir.AxisListType.X,
                )
                cc_in = dpool.tile([f, 2], F32, tag="cc_in")
                cc_out = dpool.tile([f, 2], F32, tag="cc_out")
                nc.gpsimd.dma_start(cc_in[:], stats_loc[:])
                if mode == "nocc":
                    nc.gpsimd.dma_start(cc_out[:], cc_in[:])
                else:
                    nc.gpsimd.collective_compute(
                        "AllReduce",
                        mybir.AluOpType.add,
                        replica_groups=[list(range(n_cores))],
                        ins=[cc_in.opt()],
                        outs=[cc_out.opt()],
                    )
                stats_tot = cpool.tile(
                    [f, 2], F32, tag="stats_tot", bufs=2, name="stats_tot"
                )
                nc.gpsimd.dma_start(stats_tot[:], cc_out[:])
                st["stats_tot"] = stats_tot

            def emit_tail(st):
                """BN math + ReLU apply + untranspose + store (pass st)."""
                stats_tot = st["stats_tot"]
                sc_sb = cpool.tile([f, 12], F32, tag="sc", bufs=2)
                mean = sc_sb[:, 0:1]
                es2 = sc_sb[:, 1:2]
                msq = sc_sb[:, 2:3]
                var = sc_sb[:, 3:4]
                std = sc_sb[:, 4:5]
                rstd = sc_sb[:, 5:6]
                scl = sc_sb[:, 6:7]
                tmp = sc_sb[:, 7:8]
                shf = sc_sb[:, 8:9]
                varp = sc_sb[:, 9:10]
                nc.vector.tensor_scalar_mul(mean, stats_tot[:, 0:1], inv_count)
                nc.vector.tensor_scalar_mul(es2, stats_tot[:, 1:2], inv_count)
                nc.vector.tensor_mul(msq, mean, mean)
                nc.vector.tensor_sub(var, es2, msq)
                nc.vector.tensor_scalar_add(varp, var, BN_EPS)
                nc.scalar.activation(std, varp, mybir.ActivationFunctionType.Sqrt)
                nc.vector.reciprocal(rstd, std)
                nc.vector.tensor_mul(scl, gamma_sb[:], rstd)
                nc.vector.tensor_mul(tmp, mean, scl)
                nc.vector.tensor_sub(shf, beta_sb[:], tmp)

                # --- apply BN+ReLU, untranspose, store (ACT HWDGE ring) ---
                # Block-row untranspose: out partition p holds rows
                # 16p..16p+15, so the store is 4 KiB contiguous per
                # partition (no sub-512B RMW-penalized descriptors).
                for b in range(b_pc):
                    zr_sb = iopool.tile([f, n], F32, tag="zr")
                    nc.scalar.activation(
                        zr_sb[:],
                        st["zq"][b][:],
                        mybir.ActivationFunctionType.Relu,
                        bias=shf,
                        scale=scl,
                    )
                    # zrr[p, t, q] = zr[p, q*16 + t]  (row i = 16q + t)
                    zrr = zr_sb.rearrange("p (q t) -> p t q", t=NT)
                    out_sb = iopool.tile([128, NT, f], F32, tag="out")
                    for t in range(NT):
                        bpsum = ppool.tile([128, f], F32, tag="bpsum")
                        nc.tensor.transpose(bpsum[:], zrr[:, t, :], i64_sb[:])
                        nc.vector.tensor_copy(out_sb[:, t, :], bpsum[:])
                    nc.scalar.dma_start(
                        out[b].rearrange("(p t) f -> p t f", p=128), out_sb[:]
                    )

            if mode == "dmaonly":
                for _ in range(reps):
                    dma_only_pass()
            else:
                groups = [(b, g) for b in range(b_pc) for g in range(NG)]
                prev = None
                for _ in range(reps):
                    st = {"xt": [], "xn": [], "zq": []}
                    emit_prefetch(st)
                    for idx, (b, g) in enumerate(groups):
                        emit_group(st, b, g)
                        if idx + 1 == tail_at and prev is not None:
                            emit_tail(prev)
                            prev = None
                    emit_stats_cc(st)
                    if prev is not None:
                        # tail_at > number of groups: emit late
                        emit_tail(prev)
                    prev = st
                emit_tail(prev)

    nc.compile()
    return nc


def make_in_maps(node_feats, edge_feats, W, gamma, beta, n_cores=N_CORES):
    b, n, f = node_feats.shape
    b_pc = b // n_cores
    nt = n // 128
    node_feats = np.asarray(node_feats, dtype=np.float32)
    edge_feats = np.asarray(edge_feats, dtype=np.float32)
    wt = np.ascontiguousarray(np.asarray(W, dtype=np.float32).T)
    gamma = np.asarray(gamma, dtype=np.float32).reshape(f, 1)
    beta = np.asarray(beta, dtype=np.float32).reshape(f, 1)
    i128 = np.eye(128, dtype=np.float32).astype(ml_dtypes.bfloat16)
    i64 = np.eye(f, dtype=np.float32)
    in_maps = []
    for c in range(n_cores):
        sl = slice(c * b_pc, (c + 1) * b_pc)
        xs = node_feats[sl]
        xnp = (xs / np.float32(n)).astype(ml_dtypes.bfloat16)
        # [b_pc, n, f] -> [b_pc, 128, nt, f]: partition p holds row jt*128+p
        xnp = np.ascontiguousarray(
            xnp.reshape(b_pc, nt, 128, f).transpose(0, 2, 1, 3)
        )
        in_maps.append(
            {
                "edge": edge_feats[sl],
                "xt": np.ascontiguousarray(xs.transpose(0, 2, 1)),
                "xn": xnp,
                "wt": wt,
                "gamma": gamma,
                "beta": beta,
                "i128": i128,
                "i64": i64,
            }
        )
    return in_maps


_NC_CACHE = {}


def _get_nc(key=(N_CORES, B_PC, N, F)):
    if key not in _NC_CACHE:
        _NC_CACHE[key] = build_nc(*key)
    return _NC_CACHE[key]


def kernel(node_feats, edge_feats, W, gamma, beta):
    node_feats = np.asarray(node_feats)
    edge_feats = np.asarray(edge_feats)
    b, n, f = node_feats.shape
    n_cores = N_CORES
    b_pc = b // n_cores
    nc = _get_nc((n_cores, b_pc, n, f))
    in_maps = make_in_maps(node_feats, edge_feats, W, gamma, beta, n_cores)
    res = run_bass_kernel_spmd(nc, in_maps, list(range(n_cores)))
    outs = [res.results[c]["out"] for c in range(n_cores)]
    return np.concatenate(outs, axis=0).astype(np.float32)


# revision 17
# speedup vs baseline: 1.1553x; 1.0112x over previous
"""Trainium2 Bass kernel for NodeUpdateNetwork-style GNN message passing.

out = relu(BN((x + ((sim - dsim) @ x) / N) @ W.T))  with sync-BN over (B, N).

Sharding: data-parallel over batch across 8 NeuronCores (2 batches/core);
W/gamma/beta replicated; BN statistics all-reduced across cores in-kernel.

The on-chip pipeline keeps the feature dimension on partitions ("transposed
space") so that BN reduces run along the free axis and the BN+ReLU apply is a
single per-partition scalar-engine activation:
  - stream sim/dsim row-stripes [128, N] fp32 (contiguous HBM reads)
  - DVE: diff = sim - dsim (bf16 out)
  - PE: transpose 128x128 diff tiles (identity matmul, bf16)
  - PE: aggT[f, i] += (x/N)[j, f]^T-contracted with diffT[j, i]
  - yT = aggT + xT ; zT = W @ yT (fp32) ; BN stats; AllReduce; apply; untranspose

v2: software-pipelined across reps — the post-collective tail of pass r-1
(BN math, ReLU apply, untranspose, output stores) is emitted after the first
few stream groups of pass r, so in steady state the AllReduce latency and the
apply phase hide completely under the next pass's edge streaming. Queue
hygiene keeps the SP ring free for edge loads: collective bounce DMAs ride
the gpsimd (Pool) queue, output stores ride the scalar (ACT) HWDGE ring.
"""

import sys

if "/opt/trn_rl_repo" not in sys.path:
    sys.path.insert(0, "/opt/trn_rl_repo")

import numpy as np
import ml_dtypes

import concourse.bacc as bacc
import concourse.mybir as mybir
import concourse.tile as tile
from concourse.bass_utils import run_bass_kernel_spmd

N_CORES = 8
B, N, F = 16, 2048, 64
B_PC = B // N_CORES
BN_EPS = 1e-5
BF16 = mybir.dt.bfloat16
F32 = mybir.dt.float32


def build_nc(
    n_cores=N_CORES, b_pc=B_PC, n=N, f=F, b_total=None, reps=1, mode="full",
    tail_at=3,
):
    """Build the per-core Bass program (same program on every core).

    reps > 1 unrolls the whole computation multiple times (for timing-slope
    measurements: HW time per pass = (t(reps=R) - t(reps=1)) / (R - 1)).
    mode: "full" | "nocc" (collective replaced by local dram copy, timing
    only) | "dmaonly" (edge stream loads only, timing only).
    tail_at: how many stream groups of pass r are emitted before the tail of
    pass r-1 (software pipelining depth for hiding the collective latency).
    """
    assert f == 64
    if b_total is None:
        b_total = n_cores * b_pc
    NT = n // 128                      # number of 128-wide j tiles
    IB = min(4, NT)                    # i-blocks (128 rows) per group
    GW = IB * 128                      # group width along i (<= 512)
    NG = n // GW                       # groups per batch
    inv_count = 1.0 / (b_total * n)

    nc = bacc.Bacc(
        "TRN2", target_bir_lowering=False, debug=False, num_devices=n_cores
    )

    edge = nc.dram_tensor("edge", [b_pc, 2, n, n], F32, kind="ExternalInput").ap()
    xt = nc.dram_tensor("xt", [b_pc, f, n], F32, kind="ExternalInput").ap()
    # xn is pre-laid-out host-side as [128, NT, f] per batch (contiguous DMA)
    xn = nc.dram_tensor("xn", [b_pc, 128, NT, f], BF16, kind="ExternalInput").ap()
    wt = nc.dram_tensor("wt", [f, f], BF16, kind="ExternalInput").ap()
    gamma = nc.dram_tensor("gamma", [f, 1], F32, kind="ExternalInput").ap()
    beta = nc.dram_tensor("beta", [f, 1], F32, kind="ExternalInput").ap()
    i128 = nc.dram_tensor("i128", [128, 128], BF16, kind="ExternalInput").ap()
    i64 = nc.dram_tensor("i64", [f, f], BF16, kind="ExternalInput").ap()
    out = nc.dram_tensor("out", [b_pc, n, f], F32, kind="ExternalOutput").ap()

    with tile.TileContext(nc) as tc:
        with (
            tc.tile_pool(name="const", bufs=1) as cpool,
            tc.tile_pool(name="io", bufs=2) as iopool,
            tc.tile_pool(name="zq", bufs=2 * b_pc) as zqpool,
            tc.tile_pool(name="stream", bufs=3) as spool,
            tc.tile_pool(name="blk", bufs=2) as bpool,
            tc.tile_pool(name="psum", bufs=2, space="PSUM") as ppool,
            tc.tile_pool(name="dram", bufs=2, space="DRAM") as dpool,
        ):
            # --- constants ---
            i128_sb = cpool.tile([128, 128], BF16)
            nc.sync.dma_start(i128_sb[:], i128[:])
            i64_sb = cpool.tile([f, f], BF16)
            nc.sync.dma_start(i64_sb[:], i64[:])
            wt_sb = cpool.tile([f, f], BF16)
            nc.sync.dma_start(wt_sb[:], wt[:])
            gamma_sb = cpool.tile([f, 1], F32)
            nc.sync.dma_start(gamma_sb[:], gamma[:])
            beta_sb = cpool.tile([f, 1], F32)
            nc.sync.dma_start(beta_sb[:], beta[:])

            def dma_only_pass():
                # dummy consumer so bacc/walrus DCE keeps the loads
                dum = cpool.tile([128, 2], F32, tag="dum")
                for b in range(b_pc):
                    for g in range(NG):
                        for ib in range(IB):
                            i0 = g * GW + ib * 128
                            sim_sb = spool.tile([128, n], F32, tag="sim")
                            nc.sync.dma_start(
                                sim_sb[:], edge[b, 0, i0 : i0 + 128, :]
                            )
                            dsim_sb = spool.tile([128, n], F32, tag="dsim")
                            nc.sync.dma_start(
                                dsim_sb[:], edge[b, 1, i0 : i0 + 128, :]
                            )
                            nc.vector.reduce_sum(
                                dum[:, 0:1], sim_sb[:, 0:4],
                                axis=mybir.AxisListType.X,
                            )
                            nc.vector.reduce_sum(
                                dum[:, 1:2], dsim_sb[:, 0:4],
                                axis=mybir.AxisListType.X,
                            )
                nc.sync.dma_start(out[0, 0:128, 0:2], dum[:])

            def emit_prefetch(st):
                """Load per-batch node features for a pass (SP ring)."""
                for b in range(b_pc):
                    xt_sb = iopool.tile([f, n], F32, tag="xt", bufs=b_pc)
                    nc.sync.dma_start(xt_sb[:], xt[b])
                    xn_sb = iopool.tile([128, NT, f], BF16, tag="xn", bufs=b_pc)
                    nc.sync.dma_start(xn_sb[:], xn[b])
                    zq_sb = zqpool.tile([f, n], F32, tag="zq")
                    st["xt"].append(xt_sb)
                    st["xn"].append(xn_sb)
                    st["zq"].append(zq_sb)
                st["stats"] = cpool.tile(
                    [f, b_pc * NG, 2], F32, tag="stats", bufs=2, name="stats_sb"
                )

            def emit_group_a(st, b, g):
                """Stage A: loads, subs, PE transposes, PSUM->SBUF copies.

                Returns the dT tile for the deferred stage B. Copies all ride
                the ACT queue so the DVE queue stays pure subs (edge-buffer
                WAR gates on the subs; any slow op queued between subs stalls
                the edge stream).
                """
                # --- load group stripes (sim+dsim paired, 2 MiB) + subtract ---
                diff_all = bpool.tile([128, IB, n], BF16, tag="diff")
                for ib in range(IB):
                    i0 = g * GW + ib * 128
                    sd_sb = spool.tile([128, 2, n], F32, tag="sd")
                    nc.sync.dma_start(
                        sd_sb[:],
                        edge[b, :, i0 : i0 + 128, :].rearrange("s p n -> p s n"),
                    )
                    nc.vector.tensor_sub(
                        diff_all[:, ib, :], sd_sb[:, 0, :], sd_sb[:, 1, :]
                    )

                # --- transpose diff tiles: dT[j, i] = diff[i, j] ---
                # ib-major so the PE starts on stripe 0 the moment its sub
                # lands (keeps the PE smoothly busy -> HAM stays at 2.4 GHz)
                dT_all = bpool.tile([128, NT, GW], BF16, tag="dT")
                for ib in range(IB):
                    tpsum = ppool.tile([128, NT, 128], BF16, tag="tpsum")
                    for jt in range(NT):
                        nc.tensor.transpose(
                            tpsum[:, jt, :],
                            diff_all[:, ib, jt * 128 : (jt + 1) * 128],
                            i128_sb[:],
                        )
                    nc.scalar.copy(
                        dT_all[:, :, ib * 128 : (ib + 1) * 128], tpsum[:]
                    )
                return dT_all

            def emit_group_b(st, b, g, dT_all):
                """Stage B: agg matmuls, yT, zT, z stash, BN partials.

                Emitted one group late so the PE queue never sits on the agg
                matmuls waiting for stage A's copies."""
                xt_sb, xn_sb, zq_sb = st["xt"][b], st["xn"][b], st["zq"][b]
                # --- aggT[f, i] = sum_j (x/N)[j, f] * diff[i, j] ---
                aggT = ppool.tile([f, GW], F32, tag="agg", bufs=1)
                for jt in range(NT):
                    nc.tensor.matmul(
                        aggT[:],
                        xn_sb[:, jt, :],
                        dT_all[:, jt, :],
                        start=(jt == 0),
                        stop=(jt == NT - 1),
                    )

                # --- yT = aggT + xT ; zT = W @ yT (bf16 operands) ---
                yT_sb = bpool.tile([f, GW], BF16, tag="yT")
                nc.vector.tensor_add(
                    yT_sb[:], aggT[:], xt_sb[:, g * GW : (g + 1) * GW]
                )
                zT = ppool.tile([f, GW], F32, tag="zT", bufs=1)
                nc.tensor.matmul(
                    zT[:], wt_sb[:], yT_sb[:], start=True, stop=True
                )

                # stash z and accumulate BN partial sums
                nc.scalar.copy(zq_sb[:, g * GW : (g + 1) * GW], zT[:])
                gi = b * NG + g
                nc.vector.reduce_sum(
                    st["stats"][:, gi, 0:1], zT[:], axis=mybir.AxisListType.X
                )
                sq_sb = bpool.tile([f, GW], F32, tag="sq")
                nc.scalar.activation(
                    sq_sb[:],
                    zT[:],
                    mybir.ActivationFunctionType.Square,
                    accum_out=st["stats"][:, gi, 1:2],
                )

            def emit_stats_cc(st):
                """Local stats -> global stats (sync-BN all-reduce).

                All bounce DMAs + the collective ride the gpsimd (Pool)
                queue so no stream engine ever waits on the collective.
                """
                stats_loc = cpool.tile([f, 2], F32, tag="stats_loc", bufs=2)
                nc.vector.reduce_sum(
                    stats_loc[:],
                    st["stats"][:].rearrange("p g s -> p s g"),
                    axis=mybir.AxisListType.X,
                )
                cc_in = dpool.tile([f, 2], F32, tag="cc_in")
                cc_out = dpool.tile([f, 2], F32, tag="cc_out")
                nc.gpsimd.dma_start(cc_in[:], stats_loc[:])
                if mode == "nocc":
                    nc.gpsimd.dma_start(cc_out[:], cc_in[:])
                else:
                    nc.gpsimd.collective_compute(
                        "AllReduce",
                        mybir.AluOpType.add,
                        replica_groups=[list(range(n_cores))],
                        ins=[cc_in.opt()],
                        outs=[cc_out.opt()],
                    )
                stats_tot = cpool.tile(
                    [f, 2], F32, tag="stats_tot", bufs=2, name="stats_tot"
                )
                nc.gpsimd.dma_start(stats_tot[:], cc_out[:])
                st["stats_tot"] = stats_tot

            def emit_tail_stats(st):
                """BN math for pass st (tiny DVE/ACT ops)."""
                stats_tot = st["stats_tot"]
                sc_sb = cpool.tile([f, 12], F32, tag="sc", bufs=2)
                st["sc"] = sc_sb
                mean = sc_sb[:, 0:1]
                es2 = sc_sb[:, 1:2]
                msq = sc_sb[:, 2:3]
                var = sc_sb[:, 3:4]
                std = sc_sb[:, 4:5]
                rstd = sc_sb[:, 5:6]
                scl = sc_sb[:, 6:7]
                tmp = sc_sb[:, 7:8]
                shf = sc_sb[:, 8:9]
                varp = sc_sb[:, 9:10]
                nc.vector.tensor_scalar_mul(mean, stats_tot[:, 0:1], inv_count)
                nc.vector.tensor_scalar_mul(es2, stats_tot[:, 1:2], inv_count)
                nc.vector.tensor_mul(msq, mean, mean)
                nc.vector.tensor_sub(var, es2, msq)
                nc.vector.tensor_scalar_add(varp, var, BN_EPS)
                nc.scalar.activation(std, varp, mybir.ActivationFunctionType.Sqrt)
                nc.vector.reciprocal(rstd, std)
                nc.vector.tensor_mul(scl, gamma_sb[:], rstd)
                nc.vector.tensor_mul(tmp, mean, scl)
                nc.vector.tensor_sub(shf, beta_sb[:], tmp)

            def emit_tail_apply(st, b):
                """BN+ReLU apply, untranspose, store for one batch of pass st.

                Block-row untranspose: out partition p holds rows 16p..16p+15,
                so the store is 4 KiB contiguous per partition (no sub-512B
                RMW-penalized descriptors). zr/bpsum are bf16 so the 16 PE
                untransposes are short (no fp32 4x matmul passes mid-stream).
                """
                sc_sb = st["sc"]
                scl = sc_sb[:, 6:7]
                shf = sc_sb[:, 8:9]
                zr_sb = iopool.tile([f, n], BF16, tag="zr")
                nc.scalar.activation(
                    zr_sb[:],
                    st["zq"][b][:],
                    mybir.ActivationFunctionType.Relu,
                    bias=shf,
                    scale=scl,
                )
                # zrr[p, t, q] = zr[p, q*16 + t]  (row i = 16q + t)
                zrr = zr_sb.rearrange("p (q t) -> p t q", t=NT)
                out_sb = iopool.tile([128, NT, f], F32, tag="out")
                for t in range(NT):
                    bpsum = ppool.tile([128, f], BF16, tag="bpsum")
                    nc.tensor.transpose(bpsum[:], zrr[:, t, :], i64_sb[:])
                    nc.vector.tensor_copy(out_sb[:, t, :], bpsum[:])
                nc.scalar.dma_start(
                    out[b].rearrange("(p t) f -> p t f", p=128), out_sb[:]
                )

            if mode == "dmaonly":
                for _ in range(reps):
                    dma_only_pass()
            else:
                groups = [(b, g) for b in range(b_pc) for g in range(NG)]

                def drain_tail(pst):
                    if pst is None:
                        return
                    if "sc" not in pst:
                        emit_tail_stats(pst)
                    for bb in range(pst["applied"], b_pc):
                        emit_tail_apply(pst, bb)
                    pst["applied"] = b_pc

                prev = None
                for _ in range(reps):
                    st = {"xt": [], "xn": [], "zq": [], "applied": 0}
                    emit_prefetch(st)
                    pending_b = None
                    for idx, (b, g) in enumerate(groups):
                        dT = emit_group_a(st, b, g)
                        if pending_b is not None:
                            emit_group_b(st, *pending_b)
                        pending_b = (b, g, dT)
                        if prev is not None:
                            if idx + 1 == tail_at:
                                emit_tail_stats(prev)
                                emit_tail_apply(prev, 0)
                                prev["applied"] = 1
                            elif idx + 1 == tail_at + 2:
                                drain_tail(prev)
                                prev = None
                    emit_group_b(st, *pending_b)
                    emit_stats_cc(st)
                    drain_tail(prev)
                    prev = st
                drain_tail(prev)

    nc.compile()
    return nc


def make_in_maps(node_feats, edge_feats, W, gamma, beta, n_cores=N_CORES):
    b, n, f = node_feats.shape
    b_pc = b // n_cores
    nt = n // 128
    node_feats = np.asarray(node_feats, dtype=np.float32)
    edge_feats = np.asarray(edge_feats, dtype=np.float32)
    wt = np.ascontiguousarray(np.asarray(W, dtype=np.float32).T).astype(
        ml_dtypes.bfloat16
    )
    gamma = np.asarray(gamma, dtype=np.float32).reshape(f, 1)
    beta = np.asarray(beta, dtype=np.float32).reshape(f, 1)
    i128 = np.eye(128, dtype=np.float32).astype(ml_dtypes.bfloat16)
    i64 = np.eye(f, dtype=np.float32).astype(ml_dtypes.bfloat16)
    in_maps = []
    for c in range(n_cores):
        sl = slice(c * b_pc, (c + 1) * b_pc)
        xs = node_feats[sl]
        xnp = (xs / np.float32(n)).astype(ml_dtypes.bfloat16)
        # [b_pc, n, f] -> [b_pc, 128, nt, f]: partition p holds row jt*128+p
        xnp = np.ascontiguousarray(
            xnp.reshape(b_pc, nt, 128, f).transpose(0, 2, 1, 3)
        )
        in_maps.append(
            {
                "edge": edge_feats[sl],
                "xt": np.ascontiguousarray(xs.transpose(0, 2, 1)),
                "xn": xnp,
                "wt": wt,
                "gamma": gamma,
                "beta": beta,
                "i128": i128,
                "i64": i64,
            }
        )
    return in_maps


_NC_CACHE = {}


def _get_nc(key=(N_CORES, B_PC, N, F)):
    if key not in _NC_CACHE:
        _NC_CACHE[key] = build_nc(*key)
    return _NC_CACHE[key]


def kernel(node_feats, edge_feats, W, gamma, beta):
    node_feats = np.asarray(node_feats)
    edge_feats = np.asarray(edge_feats)
    b, n, f = node_feats.shape
    n_cores = N_CORES
    b_pc = b // n_cores
    nc = _get_nc((n_cores, b_pc, n, f))
    in_maps = make_in_maps(node_feats, edge_feats, W, gamma, beta, n_cores)
    res = run_bass_kernel_spmd(nc, in_maps, list(range(n_cores)))
    outs = [res.results[c]["out"] for c in range(n_cores)]
    return np.concatenate(outs, axis=0).astype(np.float32)


# revision 22
# speedup vs baseline: 1.2067x; 1.0445x over previous
"""Trainium2 Bass kernel for NodeUpdateNetwork-style GNN message passing.

out = relu(BN((x + ((sim - dsim) @ x) / N) @ W.T))  with sync-BN over (B, N).

Sharding: data-parallel over batch across 8 NeuronCores (2 batches/core);
W/gamma/beta replicated; BN statistics all-reduced across cores in-kernel.

The on-chip pipeline keeps the feature dimension on partitions ("transposed
space") so that BN reduces run along the free axis and the BN+ReLU apply is a
single per-partition scalar-engine activation:
  - stream sim/dsim row-stripes [128, N] fp32 (contiguous HBM reads)
  - DVE: diff = sim - dsim (bf16 out)
  - PE: transpose 128x128 diff tiles (identity matmul, bf16)
  - PE: aggT[f, i] += (x/N)[j, f]^T-contracted with diffT[j, i]
  - yT = aggT + xT ; zT = W @ yT (fp32) ; BN stats; AllReduce; apply; untranspose

v2: software-pipelined across reps — the post-collective tail of pass r-1
(BN math, ReLU apply, untranspose, output stores) is emitted after the first
few stream groups of pass r, so in steady state the AllReduce latency and the
apply phase hide completely under the next pass's edge streaming. Queue
hygiene keeps the SP ring free for edge loads: collective bounce DMAs ride
the gpsimd (Pool) queue, output stores ride the scalar (ACT) HWDGE ring.
"""

import sys

if "/opt/trn_rl_repo" not in sys.path:
    sys.path.insert(0, "/opt/trn_rl_repo")

import numpy as np
import ml_dtypes

import concourse.bacc as bacc
import concourse.mybir as mybir
import concourse.tile as tile
from concourse.bass_utils import run_bass_kernel_spmd

N_CORES = 8
B, N, F = 16, 2048, 64
B_PC = B // N_CORES
BN_EPS = 1e-5
BF16 = mybir.dt.bfloat16
F32 = mybir.dt.float32


def build_nc(
    n_cores=N_CORES, b_pc=B_PC, n=N, f=F, b_total=None, reps=1, mode="full",
    tail_at=3,
):
    """Build the per-core Bass program (same program on every core).

    reps > 1 unrolls the whole computation multiple times (for timing-slope
    measurements: HW time per pass = (t(reps=R) - t(reps=1)) / (R - 1)).
    mode: "full" | "nocc" (collective replaced by local dram copy, timing
    only) | "dmaonly" (edge stream loads only, timing only).
    tail_at: how many stream groups of pass r are emitted before the tail of
    pass r-1 (software pipelining depth for hiding the collective latency).
    """
    assert f == 64
    if b_total is None:
        b_total = n_cores * b_pc
    NT = n // 128                      # number of 128-wide j tiles
    IB = min(4, NT)                    # i-blocks (128 rows) per group
    GW = IB * 128                      # group width along i (<= 512)
    NG = n // GW                       # groups per batch
    inv_count = 1.0 / (b_total * n)

    nc = bacc.Bacc(
        "TRN2", target_bir_lowering=False, debug=False, num_devices=n_cores
    )

    edge = nc.dram_tensor("edge", [b_pc, 2, n, n], F32, kind="ExternalInput").ap()
    xt = nc.dram_tensor("xt", [b_pc, f, n], BF16, kind="ExternalInput").ap()
    # xn is pre-laid-out host-side as [128, NT, f] per batch (contiguous DMA)
    xn = nc.dram_tensor("xn", [b_pc, 128, NT, f], BF16, kind="ExternalInput").ap()
    wt = nc.dram_tensor("wt", [f, f], BF16, kind="ExternalInput").ap()
    gamma = nc.dram_tensor("gamma", [f, 1], F32, kind="ExternalInput").ap()
    beta = nc.dram_tensor("beta", [f, 1], F32, kind="ExternalInput").ap()
    i128 = nc.dram_tensor("i128", [128, 128], BF16, kind="ExternalInput").ap()
    i64 = nc.dram_tensor("i64", [f, f], BF16, kind="ExternalInput").ap()
    out = nc.dram_tensor("out", [b_pc, n, f], F32, kind="ExternalOutput").ap()

    with tile.TileContext(nc) as tc:
        with (
            tc.tile_pool(name="const", bufs=1) as cpool,
            tc.tile_pool(name="io", bufs=2) as iopool,
            tc.tile_pool(name="zq", bufs=2 * b_pc) as zqpool,
            tc.tile_pool(name="stream", bufs=3) as spool,
            tc.tile_pool(name="blk", bufs=2) as bpool,
            tc.tile_pool(name="psum", bufs=2, space="PSUM") as ppool,
            tc.tile_pool(name="dram", bufs=2, space="DRAM") as dpool,
        ):
            # --- constants ---
            i128_sb = cpool.tile([128, 128], BF16)
            nc.sync.dma_start(i128_sb[:], i128[:])
            i64_sb = cpool.tile([f, f], BF16)
            nc.sync.dma_start(i64_sb[:], i64[:])
            wt_sb = cpool.tile([f, f], BF16)
            nc.sync.dma_start(wt_sb[:], wt[:])
            gamma_sb = cpool.tile([f, 1], F32)
            nc.sync.dma_start(gamma_sb[:], gamma[:])
            beta_sb = cpool.tile([f, 1], F32)
            nc.sync.dma_start(beta_sb[:], beta[:])

            def dma_only_pass():
                # dummy consumer so bacc/walrus DCE keeps the loads
                dum = cpool.tile([128, 2], F32, tag="dum")
                for b in range(b_pc):
                    for g in range(NG):
                        for ib in range(IB):
                            i0 = g * GW + ib * 128
                            sim_sb = spool.tile([128, n], F32, tag="sim")
                            nc.sync.dma_start(
                                sim_sb[:], edge[b, 0, i0 : i0 + 128, :]
                            )
                            dsim_sb = spool.tile([128, n], F32, tag="dsim")
                            nc.sync.dma_start(
                                dsim_sb[:], edge[b, 1, i0 : i0 + 128, :]
                            )
                            nc.vector.reduce_sum(
                                dum[:, 0:1], sim_sb[:, 0:4],
                                axis=mybir.AxisListType.X,
                            )
                            nc.vector.reduce_sum(
                                dum[:, 1:2], dsim_sb[:, 0:4],
                                axis=mybir.AxisListType.X,
                            )
                nc.sync.dma_start(out[0, 0:128, 0:2], dum[:])

            def emit_prefetch(st):
                """Load per-batch node features for a pass (SP ring)."""
                for b in range(b_pc):
                    xt_sb = iopool.tile([f, n], BF16, tag="xt", bufs=b_pc)
                    nc.sync.dma_start(xt_sb[:], xt[b])
                    xn_sb = iopool.tile([128, NT, f], BF16, tag="xn", bufs=b_pc)
                    nc.sync.dma_start(xn_sb[:], xn[b])
                    zq_sb = zqpool.tile([f, n], BF16, tag="zq")
                    st["xt"].append(xt_sb)
                    st["xn"].append(xn_sb)
                    st["zq"].append(zq_sb)
                st["stats"] = cpool.tile(
                    [f, b_pc * NG, 2], F32, tag="stats", bufs=2, name="stats_sb"
                )

            def emit_group_a(st, b, g):
                """Stage A: loads, subs, PE transposes, PSUM->SBUF copies.

                Returns the dT tile for the deferred stage B. Copies all ride
                the ACT queue so the DVE queue stays pure subs (edge-buffer
                WAR gates on the subs; any slow op queued between subs stalls
                the edge stream).
                """
                # --- load group stripes (sim+dsim paired, 2 MiB) + subtract ---
                diff_all = bpool.tile([128, IB, n], BF16, tag="diff")
                for ib in range(IB):
                    i0 = g * GW + ib * 128
                    sd_sb = spool.tile([128, 2, n], F32, tag="sd", bufs=5)
                    nc.sync.dma_start(
                        sd_sb[:],
                        edge[b, :, i0 : i0 + 128, :].rearrange("s p n -> p s n"),
                    )
                    nc.vector.tensor_sub(
                        diff_all[:, ib, :], sd_sb[:, 0, :], sd_sb[:, 1, :]
                    )

                # --- transpose diff tiles: dT[j, i] = diff[i, j] ---
                # ib-major so the PE starts on stripe 0 the moment its sub
                # lands (keeps the PE smoothly busy -> HAM stays at 2.4 GHz)
                dT_all = bpool.tile([128, NT, GW], BF16, tag="dT")
                for ib in range(IB):
                    tpsum = ppool.tile([128, NT, 128], BF16, tag="tpsum")
                    for jt in range(NT):
                        nc.tensor.transpose(
                            tpsum[:, jt, :],
                            diff_all[:, ib, jt * 128 : (jt + 1) * 128],
                            i128_sb[:],
                        )
                    nc.scalar.copy(
                        dT_all[:, :, ib * 128 : (ib + 1) * 128], tpsum[:]
                    )
                return dT_all

            def emit_group_b(st, b, g, dT_all):
                """Stage B: agg matmuls, yT, zT, z stash, BN partials.

                Emitted one group late so the PE queue never sits on the agg
                matmuls waiting for stage A's copies."""
                xt_sb, xn_sb, zq_sb = st["xt"][b], st["xn"][b], st["zq"][b]
                # --- aggT[f, i] = sum_j (x/N)[j, f] * diff[i, j] ---
                aggT = ppool.tile([f, GW], F32, tag="agg", bufs=1)
                for jt in range(NT):
                    nc.tensor.matmul(
                        aggT[:],
                        xn_sb[:, jt, :],
                        dT_all[:, jt, :],
                        start=(jt == 0),
                        stop=(jt == NT - 1),
                    )

                # --- yT = aggT + xT ; zT = W @ yT (bf16 operands) ---
                yT_sb = bpool.tile([f, GW], BF16, tag="yT")
                nc.vector.tensor_add(
                    yT_sb[:], aggT[:], xt_sb[:, g * GW : (g + 1) * GW]
                )
                zT = ppool.tile([f, GW], F32, tag="zT", bufs=1)
                nc.tensor.matmul(
                    zT[:], wt_sb[:], yT_sb[:], start=True, stop=True
                )

                # stash z and accumulate BN partial sums
                nc.scalar.copy(zq_sb[:, g * GW : (g + 1) * GW], zT[:])
                gi = b * NG + g
                nc.vector.reduce_sum(
                    st["stats"][:, gi, 0:1], zT[:], axis=mybir.AxisListType.X
                )
                sq_sb = bpool.tile([f, GW], F32, tag="sq")
                nc.scalar.activation(
                    sq_sb[:],
                    zT[:],
                    mybir.ActivationFunctionType.Square,
                    accum_out=st["stats"][:, gi, 1:2],
                )

            def emit_stats_cc(st):
                """Local stats -> global stats (sync-BN all-reduce).

                All bounce DMAs + the collective ride the gpsimd (Pool)
                queue so no stream engine ever waits on the collective.
                """
                stats_loc = cpool.tile([f, 2], F32, tag="stats_loc", bufs=2)
                nc.vector.reduce_sum(
                    stats_loc[:],
                    st["stats"][:].rearrange("p g s -> p s g"),
                    axis=mybir.AxisListType.X,
                )
                cc_in = dpool.tile([f, 2], F32, tag="cc_in")
                cc_out = dpool.tile([f, 2], F32, tag="cc_out")
                nc.gpsimd.dma_start(cc_in[:], stats_loc[:])
                if mode == "nocc":
                    nc.gpsimd.dma_start(cc_out[:], cc_in[:])
                else:
                    nc.gpsimd.collective_compute(
                        "AllReduce",
                        mybir.AluOpType.add,
                        replica_groups=[list(range(n_cores))],
                        ins=[cc_in.opt()],
                        outs=[cc_out.opt()],
                    )
                stats_tot = cpool.tile(
                    [f, 2], F32, tag="stats_tot", bufs=2, name="stats_tot"
                )
                nc.gpsimd.dma_start(stats_tot[:], cc_out[:])
                st["stats_tot"] = stats_tot

            def emit_tail_stats(st):
                """BN math for pass st (tiny DVE/ACT ops)."""
                stats_tot = st["stats_tot"]
                sc_sb = cpool.tile([f, 12], F32, tag="sc", bufs=2)
                st["sc"] = sc_sb
                mean = sc_sb[:, 0:1]
                es2 = sc_sb[:, 1:2]
                msq = sc_sb[:, 2:3]
                var = sc_sb[:, 3:4]
                std = sc_sb[:, 4:5]
                rstd = sc_sb[:, 5:6]
                scl = sc_sb[:, 6:7]
                tmp = sc_sb[:, 7:8]
                shf = sc_sb[:, 8:9]
                varp = sc_sb[:, 9:10]
                nc.vector.tensor_scalar_mul(mean, stats_tot[:, 0:1], inv_count)
                nc.vector.tensor_scalar_mul(es2, stats_tot[:, 1:2], inv_count)
                nc.vector.tensor_mul(msq, mean, mean)
                nc.vector.tensor_sub(var, es2, msq)
                nc.vector.tensor_scalar_add(varp, var, BN_EPS)
                nc.scalar.activation(std, varp, mybir.ActivationFunctionType.Sqrt)
                nc.vector.reciprocal(rstd, std)
                nc.vector.tensor_mul(scl, gamma_sb[:], rstd)
                nc.vector.tensor_mul(tmp, mean, scl)
                nc.vector.tensor_sub(shf, beta_sb[:], tmp)

            def emit_tail_apply(st, b):
                """BN+ReLU apply, untranspose, store for one batch of pass st.

                Block-row untranspose: out partition p holds rows 16p..16p+15,
                so the store is 4 KiB contiguous per partition (no sub-512B
                RMW-penalized descriptors). zr/bpsum are bf16 so the 16 PE
                untransposes are short (no fp32 4x matmul passes mid-stream).
                """
                sc_sb = st["sc"]
                scl = sc_sb[:, 6:7]
                shf = sc_sb[:, 8:9]
                zr_sb = iopool.tile([f, n], BF16, tag="zr")
                nc.scalar.activation(
                    zr_sb[:],
                    st["zq"][b][:],
                    mybir.ActivationFunctionType.Relu,
                    bias=shf,
                    scale=scl,
                )
                # zrr[p, t, q] = zr[p, q*16 + t]  (row i = 16q + t)
                zrr = zr_sb.rearrange("p (q t) -> p t q", t=NT)
                out_sb = iopool.tile([128, NT, f], F32, tag="out")
                for t in range(NT):
                    bpsum = ppool.tile([128, f], BF16, tag="bpsum")
                    nc.tensor.transpose(bpsum[:], zrr[:, t, :], i64_sb[:])
                    nc.vector.tensor_copy(out_sb[:, t, :], bpsum[:])
                nc.scalar.dma_start(
                    out[b].rearrange("(p t) f -> p t f", p=128), out_sb[:]
                )

            if mode == "dmaonly":
                for _ in range(reps):
                    dma_only_pass()
            else:
                groups = [(b, g) for b in range(b_pc) for g in range(NG)]

                def drain_tail(pst):
                    if pst is None:
                        return
                    if "sc" not in pst:
                        emit_tail_stats(pst)
                    for bb in range(pst["applied"], b_pc):
                        emit_tail_apply(pst, bb)
                    pst["applied"] = b_pc

                prev = None
                st = {"xt": [], "xn": [], "zq": [], "applied": 0}
                emit_prefetch(st)
                for r in range(reps):
                    st_next = None
                    pending_b = None
                    for idx, (b, g) in enumerate(groups):
                        dT = emit_group_a(st, b, g)
                        if pending_b is not None:
                            emit_group_b(st, *pending_b)
                        pending_b = (b, g, dT)
                        if prev is not None:
                            if idx + 1 == tail_at:
                                emit_tail_stats(prev)
                                emit_tail_apply(prev, 0)
                                prev["applied"] = 1
                            elif idx + 1 == tail_at + 2:
                                drain_tail(prev)
                                prev = None
                        if idx + 2 == len(groups) and r + 1 < reps:
                            # prefetch the next pass's node features now so
                            # the loads don't displace its first edge stripes
                            st_next = {"xt": [], "xn": [], "zq": [], "applied": 0}
                            emit_prefetch(st_next)
                    emit_group_b(st, *pending_b)
                    emit_stats_cc(st)
                    drain_tail(prev)
                    prev = st
                    st = st_next
                drain_tail(prev)

    nc.compile()
    return nc


def make_in_maps(node_feats, edge_feats, W, gamma, beta, n_cores=N_CORES):
    b, n, f = node_feats.shape
    b_pc = b // n_cores
    nt = n // 128
    node_feats = np.asarray(node_feats, dtype=np.float32)
    edge_feats = np.asarray(edge_feats, dtype=np.float32)
    wt = np.ascontiguousarray(np.asarray(W, dtype=np.float32).T).astype(
        ml_dtypes.bfloat16
    )
    gamma = np.asarray(gamma, dtype=np.float32).reshape(f, 1)
    beta = np.asarray(beta, dtype=np.float32).reshape(f, 1)
    i128 = np.eye(128, dtype=np.float32).astype(ml_dtypes.bfloat16)
    i64 = np.eye(f, dtype=np.float32).astype(ml_dtypes.bfloat16)
    in_maps = []
    for c in range(n_cores):
        sl = slice(c * b_pc, (c + 1) * b_pc)
        xs = node_feats[sl]
        xnp = (xs / np.float32(n)).astype(ml_dtypes.bfloat16)
        # [b_pc, n, f] -> [b_pc, 128, nt, f]: partition p holds row jt*128+p
        xnp = np.ascontiguousarray(
            xnp.reshape(b_pc, nt, 128, f).transpose(0, 2, 1, 3)
        )
        in_maps.append(
            {
                "edge": edge_feats[sl],
                "xt": np.ascontiguousarray(xs.transpose(0, 2, 1)).astype(
                    ml_dtypes.bfloat16
                ),
                "xn": xnp,
                "wt": wt,
                "gamma": gamma,
                "beta": beta,
                "i128": i128,
                "i64": i64,
            }
        )
    return in_maps


_NC_CACHE = {}


def _get_nc(key=(N_CORES, B_PC, N, F)):
    if key not in _NC_CACHE:
        _NC_CACHE[key] = build_nc(*key)
    return _NC_CACHE[key]


def kernel(node_feats, edge_feats, W, gamma, beta):
    node_feats = np.asarray(node_feats)
    edge_feats = np.asarray(edge_feats)
    b, n, f = node_feats.shape
    n_cores = N_CORES
    b_pc = b // n_cores
    nc = _get_nc((n_cores, b_pc, n, f))
    in_maps = make_in_maps(node_feats, edge_feats, W, gamma, beta, n_cores)
    res = run_bass_kernel_spmd(nc, in_maps, list(range(n_cores)))
    outs = [res.results[c]["out"] for c in range(n_cores)]
    return np.concatenate(outs, axis=0).astype(np.float32)


# revision 27
# speedup vs baseline: 1.2114x; 1.0039x over previous
"""Trainium2 Bass kernel for NodeUpdateNetwork-style GNN message passing.

out = relu(BN((x + ((sim - dsim) @ x) / N) @ W.T))  with sync-BN over (B, N).

Sharding: data-parallel over batch across 8 NeuronCores (2 batches/core);
W/gamma/beta replicated; BN statistics all-reduced across cores in-kernel.

The on-chip pipeline keeps the feature dimension on partitions ("transposed
space") so that BN reduces run along the free axis and the BN+ReLU apply is a
single per-partition scalar-engine activation:
  - stream sim/dsim row-stripes [128, N] fp32 (contiguous HBM reads)
  - DVE: diff = sim - dsim (bf16 out)
  - PE: transpose 128x128 diff tiles (identity matmul, bf16)
  - PE: aggT[f, i] += (x/N)[j, f]^T-contracted with diffT[j, i]
  - yT = aggT + xT ; zT = W @ yT (fp32) ; BN stats; AllReduce; apply; untranspose

v2: software-pipelined across reps — the post-collective tail of pass r-1
(BN math, ReLU apply, untranspose, output stores) is emitted after the first
few stream groups of pass r, so in steady state the AllReduce latency and the
apply phase hide completely under the next pass's edge streaming. Queue
hygiene keeps the SP ring free for edge loads: collective bounce DMAs ride
the gpsimd (Pool) queue, output stores ride the scalar (ACT) HWDGE ring.
"""

import sys

if "/opt/trn_rl_repo" not in sys.path:
    sys.path.insert(0, "/opt/trn_rl_repo")

import numpy as np
import ml_dtypes

import concourse.bacc as bacc
import concourse.mybir as mybir
import concourse.tile as tile
from concourse.bass_utils import run_bass_kernel_spmd

N_CORES = 8
B, N, F = 16, 2048, 64
B_PC = B // N_CORES
BN_EPS = 1e-5
BF16 = mybir.dt.bfloat16
F32 = mybir.dt.float32


def build_nc(
    n_cores=N_CORES, b_pc=B_PC, n=N, f=F, b_total=None, reps=1, mode="full",
    tail_at=3,
):
    """Build the per-core Bass program (same program on every core).

    reps > 1 unrolls the whole computation multiple times (for timing-slope
    measurements: HW time per pass = (t(reps=R) - t(reps=1)) / (R - 1)).
    mode: "full" | "nocc" (collective replaced by local dram copy, timing
    only) | "dmaonly" (edge stream loads only, timing only).
    tail_at: how many stream groups of pass r are emitted before the tail of
    pass r-1 (software pipelining depth for hiding the collective latency).
    """
    assert f == 64
    if b_total is None:
        b_total = n_cores * b_pc
    NT = n // 128                      # number of 128-wide j tiles
    IB = min(4, NT)                    # i-blocks (128 rows) per group
    GW = IB * 128                      # group width along i (<= 512)
    NG = n // GW                       # groups per batch
    inv_count = 1.0 / (b_total * n)

    nc = bacc.Bacc(
        "TRN2", target_bir_lowering=False, debug=False, num_devices=n_cores
    )

    edge = nc.dram_tensor("edge", [b_pc, 2, n, n], F32, kind="ExternalInput").ap()
    xt = nc.dram_tensor("xt", [b_pc, f, n], BF16, kind="ExternalInput").ap()
    # xn is pre-laid-out host-side as [128, NT, f] per batch (contiguous DMA)
    xn = nc.dram_tensor("xn", [b_pc, 128, NT, f], BF16, kind="ExternalInput").ap()
    wt = nc.dram_tensor("wt", [f, f], BF16, kind="ExternalInput").ap()
    gamma = nc.dram_tensor("gamma", [f, 1], F32, kind="ExternalInput").ap()
    beta = nc.dram_tensor("beta", [f, 1], F32, kind="ExternalInput").ap()
    i128 = nc.dram_tensor("i128", [128, 128], BF16, kind="ExternalInput").ap()
    i64 = nc.dram_tensor("i64", [f, f], BF16, kind="ExternalInput").ap()
    out = nc.dram_tensor("out", [b_pc, n, f], F32, kind="ExternalOutput").ap()

    with tile.TileContext(nc) as tc:
        with (
            tc.tile_pool(name="const", bufs=1) as cpool,
            tc.tile_pool(name="io", bufs=2) as iopool,
            tc.tile_pool(name="zq", bufs=2 * b_pc) as zqpool,
            tc.tile_pool(name="stream", bufs=3) as spool,
            tc.tile_pool(name="blk", bufs=2) as bpool,
            tc.tile_pool(name="psum", bufs=2, space="PSUM") as ppool,
            tc.tile_pool(name="dram", bufs=2, space="DRAM") as dpool,
        ):
            # --- constants ---
            i128_sb = cpool.tile([128, 128], BF16)
            nc.sync.dma_start(i128_sb[:], i128[:])
            i64_sb = cpool.tile([f, f], BF16)
            nc.sync.dma_start(i64_sb[:], i64[:])
            wt_sb = cpool.tile([f, f], BF16)
            nc.sync.dma_start(wt_sb[:], wt[:])
            gamma_sb = cpool.tile([f, 1], F32)
            nc.sync.dma_start(gamma_sb[:], gamma[:])
            beta_sb = cpool.tile([f, 1], F32)
            nc.sync.dma_start(beta_sb[:], beta[:])

            def dma_only_pass():
                # dummy consumer so bacc/walrus DCE keeps the loads
                dum = cpool.tile([128, 2], F32, tag="dum")
                for b in range(b_pc):
                    for g in range(NG):
                        for ib in range(IB):
                            i0 = g * GW + ib * 128
                            sim_sb = spool.tile([128, n], F32, tag="sim")
                            nc.sync.dma_start(
                                sim_sb[:], edge[b, 0, i0 : i0 + 128, :]
                            )
                            dsim_sb = spool.tile([128, n], F32, tag="dsim")
                            nc.sync.dma_start(
                                dsim_sb[:], edge[b, 1, i0 : i0 + 128, :]
                            )
                            nc.vector.reduce_sum(
                                dum[:, 0:1], sim_sb[:, 0:4],
                                axis=mybir.AxisListType.X,
                            )
                            nc.vector.reduce_sum(
                                dum[:, 1:2], dsim_sb[:, 0:4],
                                axis=mybir.AxisListType.X,
                            )
                nc.sync.dma_start(out[0, 0:128, 0:2], dum[:])

            def emit_prefetch(st):
                """Load per-batch node features for a pass (SP ring)."""
                for b in range(b_pc):
                    xt_sb = iopool.tile([f, n], BF16, tag="xt", bufs=b_pc)
                    nc.sync.dma_start(xt_sb[:], xt[b])
                    xn_sb = iopool.tile([128, NT, f], BF16, tag="xn", bufs=b_pc)
                    nc.sync.dma_start(xn_sb[:], xn[b])
                    zq_sb = zqpool.tile([f, n], BF16, tag="zq")
                    st["xt"].append(xt_sb)
                    st["xn"].append(xn_sb)
                    st["zq"].append(zq_sb)
                st["stats"] = cpool.tile(
                    [f, b_pc * NG, 2], F32, tag="stats", bufs=2, name="stats_sb"
                )

            def emit_group_a(st, b, g):
                """Stage A: loads, subs, PE transposes, PSUM->SBUF copies.

                Returns the dT tile for the deferred stage B. Copies all ride
                the ACT queue so the DVE queue stays pure subs (edge-buffer
                WAR gates on the subs; any slow op queued between subs stalls
                the edge stream).
                """
                # --- load group stripes (sim+dsim paired, 2 MiB) + subtract ---
                diff_all = bpool.tile([128, IB, n], BF16, tag="diff")
                for ib in range(IB):
                    i0 = g * GW + ib * 128
                    sd_sb = spool.tile([128, 2, n], F32, tag="sd", bufs=5)
                    nc.sync.dma_start(
                        sd_sb[:],
                        edge[b, :, i0 : i0 + 128, :].rearrange("s p n -> p s n"),
                    )
                    nc.vector.tensor_sub(
                        diff_all[:, ib, :], sd_sb[:, 0, :], sd_sb[:, 1, :]
                    )

                # --- transpose diff tiles: dT[j, i] = diff[i, j] ---
                # ib-major so the PE starts on stripe 0 the moment its sub
                # lands (keeps the PE smoothly busy -> HAM stays at 2.4 GHz)
                dT_all = bpool.tile([128, NT, GW], BF16, tag="dT")
                for ib in range(IB):
                    tpsum = ppool.tile([128, NT, 128], BF16, tag="tpsum")
                    for jt in range(NT):
                        nc.tensor.transpose(
                            tpsum[:, jt, :],
                            diff_all[:, ib, jt * 128 : (jt + 1) * 128],
                            i128_sb[:],
                        )
                    nc.scalar.copy(
                        dT_all[:, :, ib * 128 : (ib + 1) * 128], tpsum[:]
                    )
                return dT_all

            def emit_group_b(st, b, g, dT_all):
                """Stage B: agg matmuls, yT, zT, z stash, BN partials.

                Emitted one group late so the PE queue never sits on the agg
                matmuls waiting for stage A's copies."""
                xt_sb, xn_sb, zq_sb = st["xt"][b], st["xn"][b], st["zq"][b]
                # --- yT[f, i] = sum_j (x/N)[j, f] * diff[i, j] + xT[f, i] ---
                # The +xT rides the same PSUM accumulation as an identity
                # matmul so the DVE queue stays pure subs (any DVE op here
                # would serialize the edge-stream subs on the PE agg).
                aggT = ppool.tile([f, GW], F32, tag="agg", bufs=1)
                for jt in range(NT):
                    nc.tensor.matmul(
                        aggT[:],
                        xn_sb[:, jt, :],
                        dT_all[:, jt, :],
                        start=(jt == 0),
                        stop=False,
                    )
                nc.tensor.matmul(
                    aggT[:],
                    i64_sb[:],
                    xt_sb[:, g * GW : (g + 1) * GW],
                    start=False,
                    stop=True,
                )

                # --- zT = W @ yT (bf16 operands) ---
                yT_sb = bpool.tile([f, GW], BF16, tag="yT")
                nc.scalar.copy(yT_sb[:], aggT[:])
                zT = ppool.tile([f, GW], F32, tag="zT", bufs=1)
                nc.tensor.matmul(
                    zT[:], wt_sb[:], yT_sb[:], start=True, stop=True
                )

                # stash z (with fused sum accum) + squared sum -> BN partials
                gi = b * NG + g
                nc.scalar.activation(
                    zq_sb[:, g * GW : (g + 1) * GW],
                    zT[:],
                    mybir.ActivationFunctionType.Copy,
                    accum_out=st["stats"][:, gi, 0:1],
                )
                sq_sb = bpool.tile([f, GW], F32, tag="sq")
                nc.scalar.activation(
                    sq_sb[:],
                    zT[:],
                    mybir.ActivationFunctionType.Square,
                    accum_out=st["stats"][:, gi, 1:2],
                )

            def emit_stats_cc(st):
                """Local stats -> global stats (sync-BN all-reduce).

                All bounce DMAs + the collective ride the gpsimd (Pool)
                queue so no stream engine ever waits on the collective.
                """
                # per-s local reduction on ACT (keeps the DVE queue free of
                # end-of-pass ops that would delay the next pass's subs)
                stats_loc = cpool.tile([f, 2], F32, tag="stats_loc", bufs=2)
                sjunk = cpool.tile([f, b_pc * NG], F32, tag="sjunk", bufs=2)
                for s in range(2):
                    nc.scalar.activation(
                        sjunk[:],
                        st["stats"][:, :, s],
                        mybir.ActivationFunctionType.Copy,
                        accum_out=stats_loc[:, s : s + 1],
                    )
                # Bounce DMAs ride the ACT HWDGE ring: SWDGE (gpsimd) DMAs
                # share SBUF descriptor-ring state with the DVE's 2-port
                # mode and can be corrupted by the constant bf16 subs.
                cc_in = dpool.tile([f, 2], F32, tag="cc_in")
                cc_out = dpool.tile([f, 2], F32, tag="cc_out")
                nc.scalar.dma_start(cc_in[:], stats_loc[:])
                if mode == "nocc":
                    nc.scalar.dma_start(cc_out[:], cc_in[:])
                else:
                    nc.gpsimd.collective_compute(
                        "AllReduce",
                        mybir.AluOpType.add,
                        replica_groups=[list(range(n_cores))],
                        ins=[cc_in.opt()],
                        outs=[cc_out.opt()],
                    )
                st["cc_out"] = cc_out

            def emit_tail_stats(st):
                """BN math for pass st (tiny DVE/ACT ops).

                The stats_tot load happens here, not in emit_stats_cc: this
                code is emitted a few groups into the NEXT pass, so the
                collective has long completed and the ACT-ring trigger's
                wait is ~zero (no mid-stream ACT stall)."""
                stats_tot = cpool.tile(
                    [f, 2], F32, tag="stats_tot", bufs=2, name="stats_tot"
                )
                nc.scalar.dma_start(stats_tot[:], st["cc_out"][:])
                sc_sb = cpool.tile([f, 12], F32, tag="sc", bufs=2)
                st["sc"] = sc_sb
                mean = sc_sb[:, 0:1]
                es2 = sc_sb[:, 1:2]
                msq = sc_sb[:, 2:3]
                var = sc_sb[:, 3:4]
                std = sc_sb[:, 4:5]
                rstd = sc_sb[:, 5:6]
                scl = sc_sb[:, 6:7]
                tmp = sc_sb[:, 7:8]
                shf = sc_sb[:, 8:9]
                varp = sc_sb[:, 9:10]
                nc.vector.tensor_scalar_mul(mean, stats_tot[:, 0:1], inv_count)
                nc.vector.tensor_scalar_mul(es2, stats_tot[:, 1:2], inv_count)
                nc.vector.tensor_mul(msq, mean, mean)
                nc.vector.tensor_sub(var, es2, msq)
                # clamp: fp32 cancellation must never drive sqrt negative
                nc.vector.tensor_scalar_max(var, var, 0.0)
                nc.vector.tensor_scalar_add(varp, var, BN_EPS)
                nc.scalar.activation(std, varp, mybir.ActivationFunctionType.Sqrt)
                nc.vector.reciprocal(rstd, std)
                nc.vector.tensor_mul(scl, gamma_sb[:], rstd)
                nc.vector.tensor_mul(tmp, mean, scl)
                nc.vector.tensor_sub(shf, beta_sb[:], tmp)

            def emit_tail_apply(st, b):
                """BN+ReLU apply, untranspose, store for one batch of pass st.

                Block-row untranspose: out partition p holds rows 16p..16p+15,
                so the store is 4 KiB contiguous per partition (no sub-512B
                RMW-penalized descriptors). zr/bpsum are bf16 so the 16 PE
                untransposes are short (no fp32 4x matmul passes mid-stream).
                """
                sc_sb = st["sc"]
                scl = sc_sb[:, 6:7]
                shf = sc_sb[:, 8:9]
                zr_sb = iopool.tile([f, n], BF16, tag="zr")
                nc.scalar.activation(
                    zr_sb[:],
                    st["zq"][b][:],
                    mybir.ActivationFunctionType.Relu,
                    bias=shf,
                    scale=scl,
                )
                # zrr[p, t, q] = zr[p, q*16 + t]  (row i = 16q + t)
                zrr = zr_sb.rearrange("p (q t) -> p t q", t=NT)
                out_sb = iopool.tile([128, NT, f], F32, tag="out")
                for t in range(NT):
                    bpsum = ppool.tile([128, f], BF16, tag="bpsum")
                    nc.tensor.transpose(bpsum[:], zrr[:, t, :], i64_sb[:])
                    if t % 2 == 0:
                        nc.vector.tensor_copy(out_sb[:, t, :], bpsum[:])
                    else:
                        nc.scalar.copy(out_sb[:, t, :], bpsum[:])
                nc.scalar.dma_start(
                    out[b].rearrange("(p t) f -> p t f", p=128), out_sb[:]
                )

            if mode == "dmaonly":
                for _ in range(reps):
                    dma_only_pass()
            else:
                groups = [(b, g) for b in range(b_pc) for g in range(NG)]

                def drain_tail(pst):
                    if pst is None:
                        return
                    if "sc" not in pst:
                        emit_tail_stats(pst)
                    for bb in range(pst["applied"], b_pc):
                        emit_tail_apply(pst, bb)
                    pst["applied"] = b_pc

                prev = None
                st = {"xt": [], "xn": [], "zq": [], "applied": 0}
                emit_prefetch(st)
                for r in range(reps):
                    st_next = None
                    pending_b = None
                    for idx, (b, g) in enumerate(groups):
                        dT = emit_group_a(st, b, g)
                        if pending_b is not None:
                            emit_group_b(st, *pending_b)
                        pending_b = (b, g, dT)
                        if prev is not None:
                            if idx + 1 == tail_at:
                                emit_tail_stats(prev)
                                emit_tail_apply(prev, 0)
                                prev["applied"] = 1
                            elif idx + 1 == tail_at + 2:
                                drain_tail(prev)
                                prev = None
                        if idx + 2 == len(groups) and r + 1 < reps:
                            # prefetch the next pass's node features now so
                            # the loads don't displace its first edge stripes
                            st_next = {"xt": [], "xn": [], "zq": [], "applied": 0}
                            emit_prefetch(st_next)
                    emit_group_b(st, *pending_b)
                    emit_stats_cc(st)
                    drain_tail(prev)
                    prev = st
                    st = st_next
                drain_tail(prev)

    nc.compile()
    return nc


def make_in_maps(node_feats, edge_feats, W, gamma, beta, n_cores=N_CORES):
    b, n, f = node_feats.shape
    b_pc = b // n_cores
    nt = n // 128
    node_feats = np.asarray(node_feats, dtype=np.float32)
    edge_feats = np.asarray(edge_feats, dtype=np.float32)
    wt = np.ascontiguousarray(np.asarray(W, dtype=np.float32).T).astype(
        ml_dtypes.bfloat16
    )
    gamma = np.asarray(gamma, dtype=np.float32).reshape(f, 1)
    beta = np.asarray(beta, dtype=np.float32).reshape(f, 1)
    i128 = np.eye(128, dtype=np.float32).astype(ml_dtypes.bfloat16)
    i64 = np.eye(f, dtype=np.float32).astype(ml_dtypes.bfloat16)
    in_maps = []
    for c in range(n_cores):
        sl = slice(c * b_pc, (c + 1) * b_pc)
        xs = node_feats[sl]
        xnp = (xs / np.float32(n)).astype(ml_dtypes.bfloat16)
        # [b_pc, n, f] -> [b_pc, 128, nt, f]: partition p holds row jt*128+p
        xnp = np.ascontiguousarray(
            xnp.reshape(b_pc, nt, 128, f).transpose(0, 2, 1, 3)
        )
        in_maps.append(
            {
                "edge": edge_feats[sl],
                "xt": np.ascontiguousarray(xs.transpose(0, 2, 1)).astype(
                    ml_dtypes.bfloat16
                ),
                "xn": xnp,
                "wt": wt,
                "gamma": gamma,
                "beta": beta,
                "i128": i128,
                "i64": i64,
            }
        )
    return in_maps


_NC_CACHE = {}


def _get_nc(key=(N_CORES, B_PC, N, F)):
    if key not in _NC_CACHE:
        _NC_CACHE[key] = build_nc(*key)
    return _NC_CACHE[key]


def kernel(node_feats, edge_feats, W, gamma, beta):
    node_feats = np.asarray(node_feats)
    edge_feats = np.asarray(edge_feats)
    b, n, f = node_feats.shape
    n_cores = N_CORES
    b_pc = b // n_cores
    nc = _get_nc((n_cores, b_pc, n, f))
    in_maps = make_in_maps(node_feats, edge_feats, W, gamma, beta, n_cores)
    res = run_bass_kernel_spmd(nc, in_maps, list(range(n_cores)))
    outs = [res.results[c]["out"] for c in range(n_cores)]
    return np.concatenate(outs, axis=0).astype(np.float32)
